# revision 10
# baseline (speedup 1.0000x reference)
"""DSAttention Trainium2 kernel (Bass/Tile), 8-core SPMD.

Problem: causal attention with per-batch scale tau and per-key bias delta.
  scores = einsum('blhe,bshe->bhls', Q, K) * tau + delta
  scores = where(mask, -1e9, scores)
  A = softmax(scores / sqrt(E), axis=-1)
  V = einsum('bhls,bshd->blhd', A, values)
  returns (V, A)

Sharding: batch*heads across 8 cores — core c handles b = c // 2 and heads
[8*(c%2), 8*(c%2)+8).  No collectives.

Device-side design (per (b,h) pair):
  - Host folds tau into Q and appends a 65th contraction row (ones in Q^T,
    delta in K^T), so scores = Q'^T.T @ K'^T in one fp32r matmul per chunk.
  - Scores accumulate in PSUM per 128-row l-tile in 256-wide chunks; chunks
    that the causal mask fully kills are never computed (outputs pre-zeroed).
  - The chunk containing the diagonal gets a precomputed [-1e9/0] pattern
    tile added (DVE) before exp.
  - exp runs on ScalarE with scale=1/sqrt(E) and accum_out giving the row
    sum in the same pass; DVE computes 1/rowsum and writes normalized A
    (f32) which streams to HBM.
  - A.V consumes the unnormalized exp tiles (fp32r): they are transposed
    128x128 on the PE (transpose-mode matmul vs identity), copied
    PSUM->SBUF, and fed as fp32r moving operands to the A.V matmul which
    accumulates V^T [64, 256] per pair of l-tiles; V^T is transposed back
    on the PE, scaled by 1/rowsum per row, and stored.
"""

import math

import numpy as np

B, L, S, H, E = 4, 1024, 1024, 16, 64
NCORES = 8
CORES_PER_B = 2
HPC = H // CORES_PER_B  # 8 (b,h) pairs per core
EP = E + 1  # contraction rows: E plus the ones/delta row
P = 128  # partitions
CH = 256  # QK matmul free-dim chunk (>=256 keeps fp32r at full rate)
NLT = L // P  # l-tiles per pair
SQ = S // CH  # s-chunks per row
SC = S // P  # 128-wide s-chunks (A.V contraction)
NEG = -1e9
SCALE = 1.0 / math.sqrt(E)

_prog_cache = {}


def _build_program(causal: bool):
    import concourse.bass as bass
    import concourse.mybir as mybir
    from concourse import bacc
    from concourse.tile import TileContext

    f32 = mybir.dt.float32
    f32r = mybir.dt.float32r
    AF = mybir.ActivationFunctionType
    ts = bass.ts

    nc = bacc.Bacc("TRN2", target_bir_lowering=False, debug=False)

    qt_d = nc.declare_dram_parameter("qt", [HPC, EP, L], f32r, isOutput=False)
    kt_d = nc.declare_dram_parameter("kt", [HPC, EP, S], f32r, isOutput=False)
    v_d = nc.declare_dram_parameter("vv", [HPC, S, E], f32r, isOutput=False)
    a_d = nc.declare_dram_parameter("a_out", [HPC, L, S], f32, isOutput=True)
    o_d = nc.declare_dram_parameter("v_out", [HPC, L, E], f32, isOutput=True)
    id128_d = nc.declare_dram_parameter("id128", [P, P], f32r, isOutput=False)
    id64_d = nc.declare_dram_parameter("id64", [64, 64], f32r, isOutput=False)
    if causal:
        mt_d = nc.declare_dram_parameter("mtiles", [2, P, CH], f32, isOutput=False)
    else:
        m_d = nc.declare_dram_parameter("maskadd", [L, S], f32, isOutput=False)

    with TileContext(nc) as tc:
        with (
            tc.tile_pool(name="const", bufs=1) as const_pool,
            tc.tile_pool(name="qk", bufs=2) as qk_pool,
            tc.tile_pool(name="aexp", bufs=12) as a_pool,
            tc.tile_pool(name="anorm", bufs=12) as an_pool,
            tc.tile_pool(name="rs", bufs=24) as rs_pool,
            tc.tile_pool(name="atsb", bufs=4) as at_pool,
            tc.tile_pool(name="vtsb", bufs=3) as vt_pool,
            tc.tile_pool(name="vosb", bufs=3) as vo_pool,
            tc.tile_pool(name="ps_sc", bufs=2, space="PSUM") as ps_sc,
            tc.tile_pool(name="ps_at", bufs=2, space="PSUM") as ps_at,
            tc.tile_pool(name="ps_va", bufs=1, space="PSUM") as ps_va,
            tc.tile_pool(name="ps_vo", bufs=1, space="PSUM") as ps_vo,
        ):
            # --- constants (host-prepared, DMA'd once) ---
            id128 = const_pool.tile([P, P], f32r, tag="id128")
            nc.sync.dma_start(out=id128, in_=id128_d[:, :])
            id64 = const_pool.tile([64, 64], f32r, tag="id64")
            nc.sync.dma_start(out=id64, in_=id64_d[:, :])
            mt = []
            if causal:
                for q in range(2):
                    m = const_pool.tile([P, CH], f32, tag=f"mt{q}")
                    nc.sync.dma_start(out=m, in_=mt_d[q])
                    mt.append(m)
            else:
                maskb = []
                for i in range(NLT):
                    m = const_pool.tile([P, S], f32, tag=f"maskb{i}")
                    nc.sync.dma_start(out=m, in_=m_d[ts(i, P), :])
                    maskb.append(m)

            copy_flip = [0]

            def psum_copy(dst, src):
                # alternate PSUM->SBUF copies between DVE and ACT
                if copy_flip[0] % 2 == 0:
                    nc.vector.tensor_copy(dst, src)
                else:
                    nc.scalar.copy(dst, src)
                copy_flip[0] += 1

            for pi in range(HPC):
                qt_sb = qk_pool.tile([EP, L], f32r, tag="qt")
                nc.sync.dma_start(out=qt_sb, in_=qt_d[pi])
                kt_sb = qk_pool.tile([EP, S], f32r, tag="kt")
                nc.sync.dma_start(out=kt_sb, in_=kt_d[pi])
                vv_sb = qk_pool.tile([P, SC * E], f32r, tag="vv")
                nc.sync.dma_start(
                    out=vv_sb.rearrange("p (c e) -> p c e", c=SC),
                    in_=v_d[pi].rearrange("(c p) e -> p c e", p=P),
                )

                aexp_tiles = []
                recips = []
                for i in range(NLT):
                    nch = (i // 2 + 1) if causal else SQ
                    W = CH * nch
                    sc_ps = ps_sc.tile([P, S], f32, tag="scores")
                    for c in range(nch):
                        nc.tensor.matmul(
                            sc_ps[:, ts(c, CH)],
                            qt_sb[:, ts(i, P)],
                            kt_sb[:, ts(c, CH)],
                            start=True, stop=True,
                        )
                    if causal:
                        q = i % 2
                        nc.vector.tensor_add(
                            sc_ps[:, ts(nch - 1, CH)],
                            sc_ps[:, ts(nch - 1, CH)],
                            mt[q],
                        )
                    else:
                        nc.vector.tensor_add(
                            sc_ps[:, 0:W], sc_ps[:, 0:W], maskb[i][:, 0:W]
                        )
                    aexp = a_pool.tile([P, S], f32r, tag="aexp")
                    rs = rs_pool.tile([P, 1], f32, tag="rs")
                    nc.scalar.activation(
                        aexp[:, 0:W], sc_ps[:, 0:W], AF.Exp,
                        scale=SCALE, accum_out=rs,
                    )
                    rc = rs_pool.tile([P, 1], f32, tag="rc")
                    nc.vector.reciprocal(rc, rs)
                    anorm = an_pool.tile([P, S], f32, tag="anorm")
                    nc.vector.tensor_scalar_mul(
                        anorm[:, 0:W], aexp[:, 0:W].bitcast(f32), rc
                    )
                    nc.sync.dma_start(
                        out=a_d[pi, ts(i, P), 0:W], in_=anorm[:, 0:W]
                    )
                    aexp_tiles.append(aexp)
                    recips.append(rc)

                # --- A @ V, two l-tiles (one 256-wide l-group) at a time ---
                for g2 in range(NLT // 2):
                    ncc = 2 * (g2 + 1) if causal else SC  # 128-wide s-chunks
                    va_ps = ps_va.tile([64, 2 * P], f32, tag="vacc")
                    for cp in range(ncc // 2):
                        at_ps = ps_at.tile([P, 2 * CH], f32r, tag="atp")
                        for k in range(2):
                            c = 2 * cp + k
                            for t in range(2):
                                lt = 2 * g2 + t
                                nc.tensor.matmul(
                                    at_ps[:, k * CH + t * P: k * CH + (t + 1) * P],
                                    aexp_tiles[lt][:, ts(c, P)],
                                    id128,
                                    is_transpose=True,
                                )
                        at_sb = at_pool.tile([P, 2 * CH], f32r, tag="atsb")
                        psum_copy(at_sb, at_ps)
                        for k in range(2):
                            c = 2 * cp + k
                            nc.tensor.matmul(
                                va_ps[0:64, :],
                                vv_sb[:, ts(c, E)],
                                at_sb[:, ts(k, CH)],
                                start=(c == 0), stop=(c == ncc - 1),
                            )
                    vt_sb = vt_pool.tile([64, 2 * P], f32r, tag="vtsb")
                    psum_copy(vt_sb, va_ps)
                    vo_ps = ps_vo.tile([P, 2 * E], f32r, tag="vop")
                    for t in range(2):
                        nc.tensor.matmul(
                            vo_ps[:, ts(t, E)],
                            vt_sb[0:64, ts(t, P)],
                            id64,
                            is_transpose=True,
                        )
                    vo_sb = vo_pool.tile([P, 2 * E], f32, tag="vosb")
                    for t in range(2):
                        nc.vector.tensor_scalar_mul(
                            vo_sb[:, ts(t, E)],
                            vo_ps[:, ts(t, E)].bitcast(f32),
                            recips[2 * g2 + t],
                        )
                    nc.sync.dma_start(
                        out=o_d[pi, 2 * P * g2: 2 * P * (g2 + 1), :].rearrange(
                            "(t p) e -> p t e", t=2
                        ),
                        in_=vo_sb.rearrange("p (t e) -> p t e", t=2),
                    )
    nc.compile()
    return nc


def _get_program(causal: bool):
    if causal not in _prog_cache:
        _prog_cache[causal] = _build_program(causal)
    return _prog_cache[causal]


def _host_prep(queries, keys, values, attn_mask, tau, delta):
    """Build per-core input maps."""
    causal_ref = np.triu(np.ones((L, S), dtype=bool), k=1)
    mask = np.asarray(attn_mask)[:, 0]  # [B, L, S]
    causal = bool(np.array_equal(mask, np.broadcast_to(causal_ref, (B, L, S))))

    queries = np.asarray(queries, dtype=np.float32)
    keys = np.asarray(keys, dtype=np.float32)
    values = np.asarray(values, dtype=np.float32)
    tau = np.asarray(tau, dtype=np.float32)
    delta = np.asarray(delta, dtype=np.float32)

    in_maps = []
    for core in range(NCORES):
        b = core // CORES_PER_B
        hs = (core % CORES_PER_B) * HPC
        # [H, E, L] slices for this core's heads
        qT = np.ascontiguousarray(
            queries[b].transpose(1, 2, 0)[hs:hs + HPC]
        ) * np.float32(tau[b, 0])
        kT = np.ascontiguousarray(keys[b].transpose(1, 2, 0)[hs:hs + HPC])
        qt = np.empty((HPC, EP, L), dtype=np.float32)
        qt[:, :E] = qT
        qt[:, E] = 1.0
        kt = np.empty((HPC, EP, S), dtype=np.float32)
        kt[:, :E] = kT
        kt[:, E] = delta[b][None, :]
        vv = np.ascontiguousarray(values[b].transpose(1, 0, 2)[hs:hs + HPC])
        im = {
            "qt": qt, "kt": kt, "vv": vv.astype(np.float32, copy=False),
            "id128": np.eye(P, dtype=np.float32),
            "id64": np.eye(64, dtype=np.float32),
        }
        if causal:
            # mtiles[q][p, y] = 0 where p + 128*q - y >= 0 else -1e9
            py = np.arange(P)[:, None] - np.arange(CH)[None, :]
            im["mtiles"] = np.stack(
                [np.where(py + q * P >= 0, 0.0, NEG) for q in range(2)]
            ).astype(np.float32)
        else:
            im["maskadd"] = np.where(mask[b], np.float32(NEG), np.float32(0.0))
        in_maps.append(im)
    return causal, in_maps


def _assemble(results):
    V = np.empty((B, L, H, E), dtype=np.float32)
    A = np.empty((B, H, L, S), dtype=np.float32)
    for core in range(NCORES):
        b = core // CORES_PER_B
        hs = (core % CORES_PER_B) * HPC
        A[b, hs:hs + HPC] = results[core]["a_out"]
        V[b, :, hs:hs + HPC, :] = results[core]["v_out"].transpose(1, 0, 2)
    return V, A


def kernel(queries, keys, values, attn_mask, tau, delta):
    from concourse.bass_utils import run_bass_kernel_spmd

    causal, in_maps = _host_prep(queries, keys, values, attn_mask, tau, delta)
    nc = _get_program(causal)
    res = run_bass_kernel_spmd(nc, in_maps, list(range(NCORES))).results
    return _assemble(res)
